# revision 1
# baseline (speedup 1.0000x reference)
"""Trainium2 Bass kernel for nn_DecoderBlock (self-attn + cross-attn + FFN).

Sharding: sequence-parallel, no collectives. 8 cores = 2 batches x 4
L-chunks of 512 tokens. Windowed self-attention (W=64) needs only a
64-row halo; cross-attention K/V are recomputed per core from the full
`mem` of that core's batch.

On-chip layout: activations are feature-major [d_partition, token_free]
so every projection is matmul(out[do,t], lhsT=W[k,do], rhs=x[k,t]) with
weights in natural [d_in, d_out] layout. Attention scores are s-major
[key_part, query_free]; softmax row-sums come from an appended ones
column in token-major V; normalization happens on the small [dv, h, t]
attention output via a DMA-broadcast reciprocal (DRAM bounce). LayerNorm
stats use ones-vector matmuls (float32r) plus a DRAM-bounce broadcast.
ALiBi bias + causal-window mask enter via per-head additive templates.
"""

import sys

sys.path.insert(0, "/opt/trn_rl_repo")

import numpy as np
import ml_dtypes

import concourse.bass as bass
import concourse.mybir as mybir
import concourse.tile as tile
from concourse import bacc
from concourse.bass_utils import run_bass_kernel_spmd

BF16 = mybir.dt.bfloat16
F32 = mybir.dt.float32
F32R = mybir.dt.float32r
AF = mybir.ActivationFunctionType
ALU = mybir.AluOpType

B, L, MEM, D, H, FF, W = 2, 2048, 2048, 1024, 16, 4096, 64
DH = D // H  # 64
P = 128
KS = D // P  # 8
NFF = FF // P  # 32
TCH = 512
HALO = 64
SK = HALO + TCH + 64  # padded self-attn key length (640)
NEG = -30000.0


def _mm(nc, out, lhsT, rhs, start, stop):
    nc.tensor.matmul(out, lhsT, rhs, start=start, stop=stop)


def build(nc):
    dt = nc.dram_tensor
    io = {}
    io["xb"] = dt("xb", [P, KS, HALO + TCH], BF16, kind="ExternalInput")
    io["xf"] = dt("xf", [P, KS, TCH], F32, kind="ExternalInput")
    io["memf"] = dt("memf", [P, KS, MEM], BF16, kind="ExternalInput")
    for n in ["swq", "swk", "swv", "cwq", "cwk", "cwv"]:
        io[n] = dt(n, [P, KS, D], BF16, kind="ExternalInput")
    for n in ["swo", "cwo"]:
        io[n] = dt(n, [P, KS, D], BF16, kind="ExternalInput")
    io["w1"] = dt("w1", [P, KS, FF], BF16, kind="ExternalInput")
    io["w2"] = dt("w2", [P, NFF, D], BF16, kind="ExternalInput")
    for n in ["sbq", "sbk", "sbo", "cbq", "cbk", "cbo", "b2",
              "g1", "be1", "g2", "be2", "g3", "be3"]:
        io[n] = dt(n, [P, KS], F32, kind="ExternalInput")
    io["b1"] = dt("b1", [P, NFF], F32, kind="ExternalInput")
    io["tmplA0"] = dt("tmplA0", [P, H, P], BF16, kind="ExternalInput")
    io["tmplA"] = dt("tmplA", [P, H, P], BF16, kind="ExternalInput")
    io["tmplB"] = dt("tmplB", [P, H, P], BF16, kind="ExternalInput")
    io["out"] = dt("out", [P, KS, TCH], F32, kind="ExternalOutput")
    for ph in ("s", "c"):
        io[f"r_d{ph}"] = dt(f"r_d{ph}", [1, H * TCH], F32)
        io[f"r2_d{ph}"] = dt(f"r2_d{ph}", [1, H * TCH], BF16)
    for i in (1, 2, 3):
        io[f"ln_d{i}"] = dt(f"ln_d{i}", [2, TCH], F32)

    with tile.TileContext(nc) as tc:
        _build_tc(nc, tc, io)
    return nc


def _build_tc(nc, tc, io):
    import contextlib

    with contextlib.ExitStack() as ctx:
        consts = ctx.enter_context(tc.tile_pool(name="consts", bufs=1))
        wp = ctx.enter_context(tc.tile_pool(name="wp", bufs=3))
        ps = ctx.enter_context(tc.tile_pool(name="ps", bufs=5, space="PSUM"))
        sm = ctx.enter_context(tc.tile_pool(name="sm", bufs=2))
        residp = ctx.enter_context(tc.tile_pool(name="residp", bufs=2))
        xqp = ctx.enter_context(tc.tile_pool(name="xqp", bufs=1))
        avp = ctx.enter_context(tc.tile_pool(name="avp", bufs=1))

        ones_f32 = consts.tile([P, 1], F32)
        nc.vector.memset(ones_f32[:], 1.0)
        ones_f = consts.tile([P, 1], F32R)
        nc.vector.tensor_copy(out=ones_f[:], in_=ones_f32[:])
        eps_t = consts.tile([P, 1], F32)
        nc.vector.memset(eps_t[:], 1e-5)
        sb = {}
        for n in ["sbq", "sbk", "sbo", "cbq", "cbk", "cbo", "b2",
                  "g1", "be1", "g2", "be2", "g3", "be3"]:
            sb[n] = consts.tile([P, KS], F32, name=n)
            nc.sync.dma_start(sb[n][:], io[n][:])
        sb["b1"] = consts.tile([P, NFF], F32, name="b1c")
        nc.sync.dma_start(sb["b1"][:], io["b1"][:])
        tA0 = consts.tile([P, H, P], BF16)
        tA = consts.tile([P, H, P], BF16)
        tB = consts.tile([P, H, P], BF16)
        nc.sync.dma_start(tA0[:], io["tmplA0"][:])
        nc.sync.dma_start(tA[:], io["tmplA"][:])
        nc.sync.dma_start(tB[:], io["tmplB"][:])

        xf_sb = residp.tile([P, KS, TCH], F32, tag="resid")
        nc.sync.dma_start(xf_sb[:], io["xf"][:])

        NSC = MEM // P  # 16

        # cross-attn K/V pools opened early so their matmuls can fill
        # PE gaps during the attention inner loops
        kp_stack = contextlib.ExitStack()
        kp = kp_stack.enter_context(tc.tile_pool(name="kp", bufs=1))

        def kv_k(g):
            k8 = kp.tile([P, 4, MEM], BF16, tag="k8", bufs=2)
            for scn in range(4):
                mc = kp.tile([P, KS, 512], BF16, tag="memc", bufs=2)
                nc.sync.dma_start(mc[:], io["memf"][:, :, bass.ts(scn, 512)])
                for do in range(4):
                    wt = wp.tile([P, KS, P], BF16, tag="wproj")
                    nc.sync.dma_start(wt[:], io["cwk"][:, :, bass.ts(g * 4 + do, P)])
                    pt = ps.tile([P, 512], F32, tag="ps")
                    for k in range(KS):
                        _mm(nc, pt[:], wt[:, k, :], mc[:, k, :], k == 0, k == KS - 1)
                    nc.scalar.activation(
                        k8[:, do, bass.ts(scn, 512)], pt[:], AF.Identity,
                        bias=sb["cbk"][:, g * 4 + do : g * 4 + do + 1], scale=1.0)
            return k8

        def kv_v(g, cp2):
            v8 = cp2.tile([P, NSC, 8 * (DH + 1)], BF16, tag="v8")
            vv = v8[:].rearrange("p s (h c) -> p s h c", c=DH + 1)
            nc.vector.memset(vv[:, :, :, DH : DH + 1], 1.0)
            wvt = wp.tile([P, KS, 512], BF16, tag="wv", bufs=1)
            nc.sync.dma_start(wvt[:], io["cwv"][:, :, bass.ts(g, 512)])
            for scn in range(4):
                mc = kp.tile([P, KS, 512], BF16, tag="memc", bufs=2)
                nc.sync.dma_start(mc[:], io["memf"][:, :, bass.ts(scn, 512)])
                for si in range(4):
                    sc = scn * 4 + si
                    pt = ps.tile([P, 512], F32, tag="ps")
                    for k in range(KS):
                        _mm(nc, pt[:], mc[:, k, bass.ts(si, P)], wvt[:, k, :],
                            k == 0, k == KS - 1)
                    dst = v8[:, sc, :].rearrange("p (h c) -> p h c",
                                                 c=DH + 1)[:, :, 0:DH]
                    nc.vector.tensor_copy(
                        out=dst, in_=pt[:].rearrange("p (h c) -> p h c", c=DH))
            return v8

        # ================= SELF-ATTENTION =================
        with tc.tile_pool(name="selfp", bufs=1) as sp, \
             tc.tile_pool(name="exps", bufs=2) as epo:
            xb_sb = sp.tile([P, KS, HALO + TCH], BF16)
            nc.sync.dma_start(xb_sb[:], io["xb"][:])

            q_sb = sp.tile([P, KS, TCH], BF16)
            for do in range(KS):
                wt = wp.tile([P, KS, P], BF16, tag="wproj")
                nc.sync.dma_start(wt[:], io["swq"][:, :, bass.ts(do, P)])
                pt = ps.tile([P, 512], F32, tag="ps")
                for k in range(KS):
                    _mm(nc, pt[:], wt[:, k, :], xb_sb[:, k, HALO:],
                        k == 0, k == KS - 1)
                nc.scalar.activation(q_sb[:, do, :], pt[:], AF.Identity,
                                     bias=sb["sbq"][:, do : do + 1], scale=1.0)

            k_sb = sp.tile([P, KS, SK], BF16)
            nc.vector.memset(k_sb[:, :, HALO + TCH :], 0.0)
            for do in range(KS):
                wt = wp.tile([P, KS, P], BF16, tag="wproj")
                nc.sync.dma_start(wt[:], io["swk"][:, :, bass.ts(do, P)])
                pt = ps.tile([P, 512], F32, tag="ps")
                pt2 = ps.tile([P, 512], F32, tag="ps")
                for k in range(KS):
                    _mm(nc, pt[:], wt[:, k, :], xb_sb[:, k, 0:512],
                        k == 0, k == KS - 1)
                for k in range(KS):
                    _mm(nc, pt2[:, :HALO], wt[:, k, :], xb_sb[:, k, 512:576],
                        k == 0, k == KS - 1)
                nc.scalar.activation(k_sb[:, do, 0:512], pt[:], AF.Identity,
                                     bias=sb["sbk"][:, do : do + 1], scale=1.0)
                nc.scalar.activation(k_sb[:, do, 512:576], pt2[:, :HALO],
                                     AF.Identity,
                                     bias=sb["sbk"][:, do : do + 1], scale=1.0)

            NSS = 5
            v_sb = sp.tile([P, NSS, H * (DH + 1)], BF16)
            vv = v_sb[:].rearrange("p s (h c) -> p s h c", c=DH + 1)
            nc.vector.memset(vv[:, :, :, DH : DH + 1], 1.0)
            for dv in range(2):
                wt = wp.tile([P, KS, 512], BF16, tag="wv", bufs=1)
                nc.sync.dma_start(wt[:], io["swv"][:, :, bass.ts(dv, 512)])
                for sc in range(NSS):
                    n_s = min(P, HALO + TCH - sc * P)
                    pt = ps.tile([P, 512], F32, tag="ps")
                    for k in range(KS):
                        _mm(nc, pt[:n_s, :], xb_sb[:, k, sc * P : sc * P + n_s],
                            wt[:, k, :], k == 0, k == KS - 1)
                    dst = v_sb[:n_s, sc, dv * 8 * (DH + 1) : (dv * 8 + 8) * (DH + 1)]
                    dst = dst.rearrange("p (h c) -> p h c", c=DH + 1)[:, :, 0:DH]
                    nc.vector.tensor_copy(
                        out=dst,
                        in_=pt[:n_s, :].rearrange("p (h c) -> p h c", c=DH))

            # cross-attn K for group 0: independent matmul work the scheduler
            # can use to fill PE gaps during the self-attention inner loop
            k8_g0 = kv_k(0)

            av2 = avp.tile([P, KS, TCH], BF16, tag="av2")
            for j in range(H // 2):
                he, ho = 2 * j, 2 * j + 1
                avs_e = sm.tile([DH, TCH], BF16, tag="avs", bufs=5)
                avs_o = sm.tile([DH, TCH], BF16, tag="avs", bufs=5)
                for tcn in range(4):
                    T0 = tcn * P
                    q_e = q_sb[0:DH, j, T0 : T0 + P]
                    q_o = q_sb[DH : 2 * DH, j, T0 : T0 + P]
                    pAe = ps.tile([P, 512], F32, tag="ps")
                    pAo = ps.tile([P, 512], F32, tag="ps")
                    pBe = ps.tile([P, 512], F32, tag="ps")
                    pBo = ps.tile([P, 512], F32, tag="ps")
                    _mm(nc, pAe[:, :P], k_sb[0:DH, j, T0 : T0 + P], q_e, True, True)
                    _mm(nc, pAo[:, :P], k_sb[DH : 2 * DH, j, T0 : T0 + P], q_o,
                        True, True)
                    _mm(nc, pBe[:, :P], k_sb[0:DH, j, T0 + P : T0 + 2 * P], q_e,
                        True, True)
                    _mm(nc, pBo[:, :P], k_sb[DH : 2 * DH, j, T0 + P : T0 + 2 * P],
                        q_o, True, True)
                    ta = tA0 if tcn == 0 else tA
                    exps = []
                    for h, pA, pB in ((he, pAe, pBe), (ho, pAo, pBo)):
                        eA_f = epo.tile([P, P], F32, tag="ef")
                        eB_f = epo.tile([P, P], F32, tag="ef2")
                        nc.vector.tensor_tensor(eA_f[:], pA[:, :P], ta[:, h, :],
                                                ALU.add)
                        nc.vector.tensor_tensor(eB_f[:], pB[:, :P], tB[:, h, :],
                                                ALU.add)
                        eA = epo.tile([P, P], BF16, tag="eb")
                        eB = epo.tile([P, P], BF16, tag="eb2")
                        nc.scalar.activation(eA[:], eA_f[:], AF.Exp)
                        nc.scalar.activation(eB[:], eB_f[:], AF.Exp)
                        exps.append((h, eA, eB))
                    for (h, eA, eB), avs in zip(exps, (avs_e, avs_o)):
                        pav = ps.tile([P, 512], F32, tag="pav", bufs=3)
                        vA = v_sb[:, tcn, h * (DH + 1) : (h + 1) * (DH + 1)]
                        _mm(nc, pav[: DH + 1, :P], vA, eA, True, False)
                        nB = min(P, HALO + TCH - (tcn + 1) * P)
                        vB = v_sb[:nB, tcn + 1, h * (DH + 1) : (h + 1) * (DH + 1)]
                        _mm(nc, pav[: DH + 1, :P], vB, eB[:nB, :], False, True)
                        nc.scalar.copy(out=avs[:, T0 : T0 + P], in_=pav[:DH, :P])
                        rs = sm.tile([P, P], F32, tag="rs", bufs=4)
                        nc.vector.tensor_copy(out=rs[DH : DH + 1, :],
                                              in_=pav[DH : DH + 1, :P])
                        nc.sync.dma_start(
                            io["r_ds"][0:1, h * TCH + T0 : h * TCH + T0 + P],
                            rs[DH : DH + 1, :])
                for h, avs in ((he, avs_e), (ho, avs_o)):
                    _head_norm(nc, sm, io, avs, h, "s", av2)

            _oproj(nc, ps, wp, sm, av2, io, "swo", sb["sbo"], xf_sb)

        cp_stack = contextlib.ExitStack()
        cp2 = cp_stack.enter_context(tc.tile_pool(name="cp2", bufs=1))
        epc = cp_stack.enter_context(tc.tile_pool(name="expc", bufs=4))
        v8_g0 = kv_v(0, cp2)

        x1_sb = residp.tile([P, KS, TCH], F32, tag="resid")
        x1b = xqp.tile([P, KS, TCH], BF16, tag="xq")
        _ln(nc, ps, sm, io, xf_sb, x1_sb, sb["g1"], sb["be1"], ones_f, eps_t, 1,
            out_bf=x1b)

        # ================= CROSS-ATTENTION =================
        q2_sb = cp2.tile([P, KS, TCH], BF16)
        for do in range(KS):
            wt = wp.tile([P, KS, P], BF16, tag="wproj")
            nc.sync.dma_start(wt[:], io["cwq"][:, :, bass.ts(do, P)])
            pt = ps.tile([P, 512], F32, tag="ps")
            for k in range(KS):
                _mm(nc, pt[:], wt[:, k, :], x1b[:, k, :], k == 0, k == KS - 1)
            nc.scalar.activation(q2_sb[:, do, :], pt[:], AF.Identity,
                                 bias=sb["cbq"][:, do : do + 1], scale=1.0)

        av2 = avp.tile([P, KS, TCH], BF16, tag="av2")
        kv = (k8_g0, v8_g0)
        for g in range(2):
            k8, v8 = kv
            for j in range(4):
                he, ho = g * 8 + 2 * j, g * 8 + 2 * j + 1
                q_e = q2_sb[0:DH, g * 4 + j, :]
                q_o = q2_sb[DH : 2 * DH, g * 4 + j, :]
                pav_e = ps.tile([P, 512], F32, tag="pav", bufs=3)
                pav_o = ps.tile([P, 512], F32, tag="pav", bufs=3)
                for sc in range(NSC):
                    pSe = ps.tile([P, 512], F32, tag="ps")
                    pSo = ps.tile([P, 512], F32, tag="ps")
                    _mm(nc, pSe[:], k8[0:DH, j, bass.ts(sc, P)], q_e, True, True)
                    _mm(nc, pSo[:], k8[DH : 2 * DH, j, bass.ts(sc, P)], q_o,
                        True, True)
                    eSe = epc.tile([P, TCH], BF16, tag="ec")
                    eSo = epc.tile([P, TCH], BF16, tag="ec2")
                    nc.scalar.activation(eSe[:], pSe[:], AF.Exp)
                    nc.scalar.activation(eSo[:], pSo[:], AF.Exp)
                    vA_e = v8[:, sc, 2 * j * (DH + 1) : (2 * j + 1) * (DH + 1)]
                    vA_o = v8[:, sc, (2 * j + 1) * (DH + 1) : (2 * j + 2) * (DH + 1)]
                    _mm(nc, pav_e[: DH + 1, :], vA_e, eSe, sc == 0, sc == NSC - 1)
                    _mm(nc, pav_o[: DH + 1, :], vA_o, eSo, sc == 0, sc == NSC - 1)
                for h, pav in ((he, pav_e), (ho, pav_o)):
                    avs = sm.tile([DH, TCH], BF16, tag="avs", bufs=5)
                    nc.scalar.copy(out=avs[:], in_=pav[:DH, :])
                    rs = sm.tile([P, TCH], F32, tag="rs2", bufs=2)
                    nc.vector.tensor_copy(out=rs[DH : DH + 1, :],
                                          in_=pav[DH : DH + 1, :])
                    nc.sync.dma_start(io["r_dc"][0:1, bass.ts(h, TCH)],
                                      rs[DH : DH + 1, :])
                    _head_norm(nc, sm, io, avs, h, "c", av2)
                if g == 0 and j == 3:
                    kv = (kv_k(1), kv_v(1, cp2))

        _oproj(nc, ps, wp, sm, av2, io, "cwo", sb["cbo"], x1_sb)
        cp_stack.close()
        kp_stack.close()

        x2_sb = residp.tile([P, KS, TCH], F32, tag="resid")
        x2b = xqp.tile([P, KS, TCH], BF16, tag="xq")
        _ln(nc, ps, sm, io, x1_sb, x2_sb, sb["g2"], sb["be2"], ones_f, eps_t, 2,
            out_bf=x2b)

        # ================= FFN =================
        with tc.tile_pool(name="ffnp", bufs=1) as fp, \
             tc.tile_pool(name="w2p", bufs=2) as w2p:
            h1 = fp.tile([P, NFF, TCH], BF16)
            for fc in range(NFF):
                wt = wp.tile([P, KS, P], BF16, tag="wproj")
                nc.sync.dma_start(wt[:], io["w1"][:, :, bass.ts(fc, P)])
                pt = ps.tile([P, 512], F32, tag="ps")
                for k in range(KS):
                    _mm(nc, pt[:], wt[:, k, :], x2b[:, k, :], k == 0, k == KS - 1)
                nc.scalar.activation(h1[:, fc, :], pt[:], AF.Gelu,
                                     bias=sb["b1"][:, fc : fc + 1], scale=1.0)
            for do in range(KS):
                wt = w2p.tile([P, NFF, P], BF16, tag="w2t")
                nc.sync.dma_start(wt[:], io["w2"][:, :, bass.ts(do, P)])
                pt = ps.tile([P, 512], F32, tag="ps")
                for k in range(NFF):
                    _mm(nc, pt[:], wt[:, k, :], h1[:, k, :], k == 0, k == NFF - 1)
                ft = sm.tile([P, TCH], F32, tag="t2k")
                nc.vector.tensor_scalar(out=ft[:], in0=pt[:],
                                        scalar1=sb["b2"][:, do : do + 1],
                                        scalar2=None, op0=ALU.add)
                nc.vector.tensor_tensor(x2_sb[:, do, :], ft[:], x2_sb[:, do, :],
                                        ALU.add)

        out_sb = residp.tile([P, KS, TCH], F32, tag="resid")
        _ln(nc, ps, sm, io, x2_sb, out_sb, sb["g3"], sb["be3"], ones_f, eps_t, 3)
        nc.sync.dma_start(io["out"][:], out_sb[:])


def _head_norm(nc, sm, io, avs, h, ph, av2):
    """Per-head softmax normalization on the staging tile, then DMA the
    normalized head into its pair-stacked position in av2."""
    rsq = sm.tile([DH, 8], F32, tag="rsq", bufs=4)
    nc.sync.dma_start(
        rsq[:], io[f"r_d{ph}"].ap()[0, bass.ts(h, TCH)].rearrange("(p c) -> p c", c=8))
    rsq2 = sm.tile([DH, 8], BF16, tag="rsq2", bufs=4)
    with nc.allow_low_precision(reason="softmax denominator broadcast in bf16"):
        nc.vector.reciprocal(out=rsq2[:], in_=rsq[:])
    nc.sync.dma_start(
        io[f"r2_d{ph}"].ap()[0, bass.ts(h, TCH)].rearrange("(p c) -> p c", c=8),
        rsq2[:])
    rcp = sm.tile([DH, TCH], BF16, tag="rcph", bufs=4)
    base = io[f"r2_d{ph}"].ap()[0, bass.ts(h, TCH)]
    srcb = bass.AP(tensor=base.tensor, offset=base.offset,
                   ap=[[0, DH]] + list(base.ap))
    nc.sync.dma_start(rcp[:], srcb)
    nc.vector.tensor_tensor(avs[:], avs[:], rcp[:], ALU.mult)
    nc.sync.dma_start(av2[(h % 2) * DH : (h % 2) * DH + DH, h // 2, :], avs[:])


def _oproj(nc, ps, wp, sm, av2, io, wo_name, bo_sb, res_sb):
    """Standard K=128 projection of the pair-stacked av2 with wo in natural
    layout; bias + residual add into res_sb in place."""
    for do in range(KS):
        wt = wp.tile([P, KS, P], BF16, tag="wproj")
        nc.sync.dma_start(wt[:], io[wo_name][:, :, bass.ts(do, P)])
        pt = ps.tile([P, 512], F32, tag="ps")
        for k in range(KS):
            _mm(nc, pt[:], wt[:, k, :], av2[:, k, :], k == 0, k == KS - 1)
        st = sm.tile([P, TCH], F32, tag="t2k")
        nc.vector.tensor_scalar(out=st[:], in0=pt[:],
                                scalar1=bo_sb[:, do : do + 1], scalar2=None,
                                op0=ALU.add)
        nc.vector.tensor_tensor(res_sb[:, do, :], st[:], res_sb[:, do, :], ALU.add)


def _ln(nc, ps, sm, io, src_sb, out_sb, g, be, ones_f, eps_t, li, out_bf=None):
    """out = LN(src) over the feature (partition) axis, feature-major."""
    onr = ones_f[:]
    psum = ps.tile([P, 512], F32, tag="ps")
    psq = ps.tile([P, 512], F32, tag="ps")
    for k in range(KS):
        xr = sm.tile([P, TCH], F32R, tag="lnxr", bufs=1)
        nc.vector.tensor_copy(out=xr[:], in_=src_sb[:, k, :])
        _mm(nc, psum[:1, :], onr, xr[:], k == 0, k == KS - 1)
        sq = sm.tile([P, TCH], F32R, tag="lnsq", bufs=1)
        nc.vector.tensor_tensor(sq[:], xr[:], xr[:], ALU.mult)
        _mm(nc, psq[:1, :], onr, sq[:], k == 0, k == KS - 1)
    mean = sm.tile([1, TCH], F32, tag="lnm")
    ex2 = sm.tile([1, TCH], F32, tag="lnv")
    nc.vector.tensor_scalar(out=mean[:], in0=psum[:1, :], scalar1=1.0 / D,
                            scalar2=None, op0=ALU.mult)
    nc.vector.tensor_scalar(out=ex2[:], in0=psq[:1, :], scalar1=1.0 / D,
                            scalar2=None, op0=ALU.mult)
    var = sm.tile([1, TCH], F32, tag="lnvar")
    nc.vector.tensor_tensor(var[:], mean[:], mean[:], ALU.mult)
    nc.vector.tensor_tensor(var[:], ex2[:], var[:], ALU.subtract)
    std = sm.tile([1, TCH], F32, tag="lnstd")
    nc.scalar.activation(std[:], var[:], AF.Sqrt, bias=eps_t[:1, :], scale=1.0)
    nc.vector.reciprocal(out=std[:], in_=std[:])
    nc.sync.dma_start(io[f"ln_d{li}"][0:1, :], mean[:1, :])
    nc.sync.dma_start(io[f"ln_d{li}"][1:2, :], std[:1, :])
    mb = sm.tile([P, TCH], F32, tag="lnb")
    rb = sm.tile([P, TCH], F32, tag="lnb")
    for i, dst in [(0, mb), (1, rb)]:
        base = io[f"ln_d{li}"].ap()[i, :]
        src = bass.AP(tensor=base.tensor, offset=base.offset,
                      ap=[[0, P]] + list(base.ap))
        nc.sync.dma_start(dst[:], src)
    for k in range(KS):
        nc.vector.tensor_tensor(out_sb[:, k, :], src_sb[:, k, :], mb[:],
                                ALU.subtract)
        nc.vector.tensor_tensor(out_sb[:, k, :], out_sb[:, k, :], rb[:], ALU.mult)
        nc.vector.tensor_scalar(out=out_sb[:, k, :], in0=out_sb[:, k, :],
                                scalar1=g[:, k : k + 1], scalar2=be[:, k : k + 1],
                                op0=ALU.mult, op1=ALU.add)
        if out_bf is not None:
            nc.vector.tensor_copy(out=out_bf[:, k, :], in_=out_sb[:, k, :])


# ======================= host side =======================

_CACHE = {}


def _fm(a):
    """[T, D] -> feature-major [128, KS, T]."""
    T = a.shape[0]
    return np.ascontiguousarray(a.T.reshape(KS, P, T).transpose(1, 0, 2))


def _wfm(w):
    """[D_in, D_out] -> [128, D_in//128, D_out]."""
    return np.ascontiguousarray(w.reshape(-1, P, w.shape[1]).transpose(1, 0, 2))


def _pbias(b):
    return np.ascontiguousarray(b.reshape(-1, P).T.astype(np.float32))


def _templates():
    slopes = (2.0 ** (-8.0 * np.arange(1, H + 1) / H)).astype(np.float64)
    sr = np.arange(P)[:, None]
    tr = np.arange(P)[None, :]
    dA = sr - tr
    dB = sr + P - tr
    A = np.where((dA >= 1) & (dA <= 64),
                 (dA - 64)[None] * slopes[:, None, None], NEG)
    Bt = np.where((dB >= 1) & (dB <= 64),
                  (dB - 64)[None] * slopes[:, None, None], NEG)
    A0 = A.copy()
    A0[:, :64, :] = NEG
    f = lambda x: np.ascontiguousarray(x.transpose(1, 0, 2).astype(np.float32))
    return f(A), f(A0), f(Bt)


def kernel(**inputs):
    bf = ml_dtypes.bfloat16
    x = np.asarray(inputs["x"], np.float32)
    mem = np.asarray(inputs["mem"], np.float32)
    g = lambda n: np.asarray(inputs[n], np.float32)

    tA, tA0, tB = _templates()
    CH = L // TCH

    shared = {
        "swq": _wfm(g("swq") / 8.0).astype(bf), "swk": _wfm(g("swk")).astype(bf),
        "swv": _wfm(g("swv")).astype(bf), "swo": _wfm(g("swo")).astype(bf),
        "cwq": _wfm(g("cwq") / 8.0).astype(bf), "cwk": _wfm(g("cwk")).astype(bf),
        "cwv": _wfm(g("cwv")).astype(bf), "cwo": _wfm(g("cwo")).astype(bf),
        "w1": _wfm(g("w1")).astype(bf), "w2": _wfm(g("w2")).astype(bf),
        "sbq": _pbias(g("sbq") / 8.0), "sbk": _pbias(g("sbk")),
        "sbo": _pbias(g("sbo") + g("sbv") @ g("swo")), "cbq": _pbias(g("cbq") / 8.0),
        "cbk": _pbias(g("cbk")), "cbo": _pbias(g("cbo") + g("cbv") @ g("cwo")),
        "b2": _pbias(g("b2")), "b1": _pbias(g("b1")),
        "g1": _pbias(g("g1")), "be1": _pbias(g("be1")),
        "g2": _pbias(g("g2")), "be2": _pbias(g("be2")),
        "g3": _pbias(g("g3")), "be3": _pbias(g("be3")),
        "tmplA": tA.astype(bf), "tmplB": tB.astype(bf),
    }
    mem_fm = [np.ascontiguousarray(_fm(mem[b]).astype(bf)) for b in range(B)]

    in_maps = []
    for core in range(8):
        b, c = core // CH, core % CH
        t0 = c * TCH
        xpad = np.zeros((HALO + TCH, D), np.float32)
        lo = max(0, t0 - HALO)
        xpad[HALO - (t0 - lo):] = x[b, lo : t0 + TCH]
        m = dict(shared)
        m["memf"] = mem_fm[b]
        m["xb"] = np.ascontiguousarray(_fm(xpad).astype(bf))
        m["xf"] = np.ascontiguousarray(_fm(x[b, t0 : t0 + TCH]))
        m["tmplA0"] = (tA0 if c == 0 else tA).astype(bf)
        in_maps.append(m)

    if "nc" not in _CACHE:
        nc = bacc.Bacc("TRN2", target_bir_lowering=False, debug=False,
                       num_devices=8)
        build(nc)
        nc.compile()
        _CACHE["nc"] = nc
    nc = _CACHE["nc"]

    res = run_bass_kernel_spmd(nc, in_maps, core_ids=list(range(8)),
                               **_CACHE.get("run_kwargs", {}))
    _CACHE["last"] = res

    y = np.empty((B, L, D), np.float32)
    for core in range(8):
        b, c = core // CH, core % CH
        o = np.asarray(res.results[core]["out"])  # [128, KS, TCH]
        y[b, c * TCH : (c + 1) * TCH, :] = o.transpose(1, 0, 2).reshape(D, TCH).T
    return y



# revision 15
# speedup vs baseline: 1.1567x; 1.1567x over previous
"""Trainium2 Bass kernel for nn_DecoderBlock (self-attn + cross-attn + FFN).

Sharding: sequence-parallel, no collectives. 8 cores = 2 batches x 4
L-chunks of 512 tokens. Windowed self-attention (W=64) needs only a
64-row halo; cross-attention K/V are recomputed per core from the full
`mem` of that core's batch.

On-chip layout: activations are feature-major [d_partition, token_free]
so every projection is matmul(out[do,t], lhsT=W[k,do], rhs=x[k,t]) with
weights in natural [d_in, d_out] layout. Attention scores are s-major
[key_part, query_free]; softmax row-sums come from an appended ones
column in token-major V. Softmax normalization is fully on-chip:
row-sum -> 1/x via exp(-ln(x)) on ACT -> gpsimd partition_broadcast ->
one DVE multiply straight into the pair-stacked av2 tile. LayerNorm
stats use ones-vector matmuls (float32r); rstd = exp(-0.5*ln(var+eps));
mean/rstd broadcast across partitions on gpsimd. ALiBi bias +
causal-window mask enter via per-pair additive score templates.

All hot matmuls are padded to full (128,128) PE tiles: partial-tile
matmuls (tile_size != (128,128)) run on a slow clock path, so score
matmuls contract over zero-padded 128-dim q vectors and AV matmuls use
128-column [v | ones | 0] stationary blocks.
"""

import sys

sys.path.insert(0, "/opt/trn_rl_repo")

import numpy as np
import ml_dtypes

import concourse.bass as bass
import concourse.mybir as mybir
import concourse.tile as tile
from concourse import bacc
from concourse.bass_utils import run_bass_kernel_spmd

BF16 = mybir.dt.bfloat16
F32 = mybir.dt.float32
F32R = mybir.dt.float32r
AF = mybir.ActivationFunctionType
ALU = mybir.AluOpType

B, L, MEM, D, H, FF, W = 2, 2048, 2048, 1024, 16, 4096, 64
DH = D // H  # 64
P = 128
KS = D // P  # 8
NFF = FF // P  # 32
TCH = 512
HALO = 64
SK = HALO + TCH + 64  # padded self-attn key length (640)
NEG = -30000.0
NBIAS = ["sbq", "sbk", "sbo", "cbq", "cbk", "cbo", "b2",
         "g1", "be1", "g2", "be2", "g3", "be3"]


def _mm(nc, out, lhsT, rhs, start, stop):
    nc.tensor.matmul(out, lhsT, rhs, start=start, stop=stop)


def build(nc):
    dt = nc.dram_tensor
    io = {}
    io["xb"] = dt("xb", [P, KS, HALO + TCH], BF16, kind="ExternalInput")
    io["xf"] = dt("xf", [P, KS, TCH], F32, kind="ExternalInput")
    io["memf"] = dt("memf", [P, KS, MEM], BF16, kind="ExternalInput")
    for n in ["swq", "swk", "swv", "cwq", "cwk", "cwv"]:
        io[n] = dt(n, [P, KS, D], BF16, kind="ExternalInput")
    for n in ["swo", "cwo"]:
        io[n] = dt(n, [P, KS, D], BF16, kind="ExternalInput")
    io["w1"] = dt("w1", [P, KS, FF], BF16, kind="ExternalInput")
    io["w2"] = dt("w2", [P, NFF, D], BF16, kind="ExternalInput")
    # all per-feature bias/scale vectors packed into one tensor: 13 KS-wide
    # blocks then b1 (NFF wide)
    io["biases"] = dt("biases", [P, 13 * KS + NFF], F32, kind="ExternalInput")
    io["tmplP0"] = dt("tmplP0", [P, KS, 4 * P], BF16, kind="ExternalInput")
    io["tmplP"] = dt("tmplP", [P, KS, 4 * P], BF16, kind="ExternalInput")
    io["out"] = dt("out", [P, KS, TCH], F32, kind="ExternalOutput")

    with tile.TileContext(nc) as tc:
        _build_tc(nc, tc, io)
    return nc


def _build_tc(nc, tc, io):
    import contextlib

    with contextlib.ExitStack() as ctx:
        consts = ctx.enter_context(tc.tile_pool(name="consts", bufs=1))
        wp = ctx.enter_context(tc.tile_pool(name="wp", bufs=3))
        ps = ctx.enter_context(tc.tile_pool(name="ps", bufs=5, space="PSUM"))
        sm = ctx.enter_context(tc.tile_pool(name="sm", bufs=2))
        residp = ctx.enter_context(tc.tile_pool(name="residp", bufs=2))
        xqp = ctx.enter_context(tc.tile_pool(name="xqp", bufs=1))
        avp = ctx.enter_context(tc.tile_pool(name="avp", bufs=1))
        qp = ctx.enter_context(tc.tile_pool(name="qp", bufs=1))

        # padded-q tiles: even head dims on partitions 0:64 (rest zero),
        # odd head dims on partitions 64:128 (rest zero). Shared between
        # the self- and cross-attention phases.
        qe = qp.tile([P, KS, TCH], BF16, tag="qe")
        qo = qp.tile([P, KS, TCH], BF16, tag="qo")
        nc.gpsimd.memset(qe[DH:P, :, :], 0.0)
        nc.gpsimd.memset(qo[0:DH, :, :], 0.0)

        biases = consts.tile([P, 13 * KS + NFF], F32)
        nc.sync.dma_start(biases[:], io["biases"][:])
        sb = {n: biases[:, i * KS : (i + 1) * KS] for i, n in enumerate(NBIAS)}
        sb["b1"] = biases[:, 13 * KS :]

        # ones128: column 0 is ones, rest zeros -> full (128,128) LN matmul
        ones128_f32 = consts.tile([P, P], F32)
        nc.vector.memset(ones128_f32[:], 0.0)
        nc.vector.memset(ones128_f32[:, 0:1], 1.0)
        ones128 = consts.tile([P, P], F32R)
        nc.vector.tensor_copy(out=ones128[:], in_=ones128_f32[:])
        eps_t = consts.tile([P, 1], F32)
        nc.vector.memset(eps_t[:], 1e-5)

        NSC = MEM // P  # 16

        # cross-attn K/V pools opened early so their matmuls can fill
        # PE gaps during the attention inner loops
        kp_stack = contextlib.ExitStack()
        kp = kp_stack.enter_context(tc.tile_pool(name="kp", bufs=1))

        def kv_k(g):
            k8 = kp.tile([P, 4, MEM], BF16, tag="k8", bufs=1)
            for scn in range(4):
                mc = kp.tile([P, KS, 512], BF16, tag="memc", bufs=2)
                nc.sync.dma_start(mc[:], io["memf"][:, :, bass.ts(scn, 512)])
                for do in range(4):
                    wt = wp.tile([P, KS, P], BF16, tag="wproj")
                    nc.sync.dma_start(wt[:], io["cwk"][:, :, bass.ts(g * 4 + do, P)])
                    pt = ps.tile([P, 512], F32, tag="ps")
                    for k in range(KS):
                        _mm(nc, pt[:], wt[:, k, :], mc[:, k, :], k == 0, k == KS - 1)
                    nc.scalar.activation(
                        k8[:, do, bass.ts(scn, 512)], pt[:], AF.Identity,
                        bias=sb["cbk"][:, g * 4 + do : g * 4 + do + 1], scale=1.0)
            return k8

        def kv_v(g, cp2):
            # per-head 65-stride blocks [v(64) | ones(1)] + a 63-col zero
            # tail; the AV lhsT window [h*65 : h*65+128] overlaps the next
            # head, whose contribution lands in psum rows 65:127 (never read)
            VW = 8 * (DH + 1) + DH - 1  # 583
            v8 = cp2.tile([P, NSC, VW], BF16, tag=f"v8_{g}")
            vv = v8[:, :, 0 : 8 * (DH + 1)].rearrange("p s (h c) -> p s h c",
                                                      c=DH + 1)
            nc.gpsimd.memset(vv[:, :, :, DH : DH + 1], 1.0)
            nc.gpsimd.memset(v8[:, :, 8 * (DH + 1) :], 0.0)
            wvt = wp.tile([P, KS, 512], BF16, tag="wv", bufs=1)
            nc.sync.dma_start(wvt[:], io["cwv"][:, :, bass.ts(g, 512)])
            for scn in range(4):
                mc = kp.tile([P, KS, 512], BF16, tag="memc", bufs=2)
                nc.sync.dma_start(mc[:], io["memf"][:, :, bass.ts(scn, 512)])
                for si in range(4):
                    sc = scn * 4 + si
                    pt = ps.tile([P, 512], F32, tag="ps")
                    for k in range(KS):
                        _mm(nc, pt[:], mc[:, k, bass.ts(si, P)], wvt[:, k, :],
                            k == 0, k == KS - 1)
                    dst = v8[:, sc, 0 : 8 * (DH + 1)].rearrange(
                        "p (h c) -> p h c", c=DH + 1)[:, :, 0:DH]
                    nc.vector.tensor_copy(
                        out=dst, in_=pt[:].rearrange("p (h c) -> p h c", c=DH))
            return v8

        # ================= SELF-ATTENTION =================
        with tc.tile_pool(name="selfp", bufs=1) as sp, \
             tc.tile_pool(name="exps", bufs=3) as epo:
            xb_sb = sp.tile([P, KS, HALO + TCH], BF16)
            nc.sync.dma_start(xb_sb[:], io["xb"][:])

            for do in range(KS):
                wt = wp.tile([P, KS, P], BF16, tag="wproj")
                nc.sync.dma_start(wt[:], io["swq"][:, :, bass.ts(do, P)])
                pt = ps.tile([P, 512], F32, tag="ps")
                for k in range(KS):
                    _mm(nc, pt[:], wt[:, k, :], xb_sb[:, k, HALO:],
                        k == 0, k == KS - 1)
                nc.scalar.activation(qe[0:DH, do, :], pt[0:DH, :], AF.Identity,
                                     bias=sb["sbq"][0:DH, do : do + 1], scale=1.0)
                nc.vector.tensor_scalar(out=qo[DH:P, do, :], in0=pt[DH:P, :],
                                        scalar1=sb["sbq"][DH:P, do : do + 1],
                                        scalar2=None, op0=ALU.add)

            tp0 = sp.tile([P, KS, 4 * P], BF16)
            tp1 = sp.tile([P, KS, 4 * P], BF16)
            nc.sync.dma_start(tp0[:], io["tmplP0"][:])
            nc.sync.dma_start(tp1[:], io["tmplP"][:])
            xf_sb = residp.tile([P, KS, TCH], F32, tag="resid")
            nc.sync.dma_start(xf_sb[:], io["xf"][:])

            k_sb = sp.tile([P, KS, SK], BF16)
            nc.vector.memset(k_sb[:, :, HALO + TCH :], 0.0)
            for do in range(KS):
                wt = wp.tile([P, KS, P], BF16, tag="wproj")
                nc.sync.dma_start(wt[:], io["swk"][:, :, bass.ts(do, P)])
                pt = ps.tile([P, 512], F32, tag="ps")
                pt2 = ps.tile([P, 512], F32, tag="ps")
                for k in range(KS):
                    _mm(nc, pt[:], wt[:, k, :], xb_sb[:, k, 0:512],
                        k == 0, k == KS - 1)
                for k in range(KS):
                    _mm(nc, pt2[:, :HALO], wt[:, k, :], xb_sb[:, k, 512:576],
                        k == 0, k == KS - 1)
                nc.scalar.activation(k_sb[:, do, 0:512], pt[:], AF.Identity,
                                     bias=sb["sbk"][:, do : do + 1], scale=1.0)
                nc.scalar.activation(k_sb[:, do, 512:576], pt2[:, :HALO],
                                     AF.Identity,
                                     bias=sb["sbk"][:, do : do + 1], scale=1.0)

            # token-major V with per-head 65-stride blocks [v | ones] and a
            # 63-col zero tail (AV lhsT windows overlap the next head)
            NSS = 5
            VW = H * (DH + 1) + DH - 1  # 1103
            v_sb = sp.tile([P, NSS, VW], BF16)
            vv = v_sb[:, :, 0 : H * (DH + 1)].rearrange("p s (h c) -> p s h c",
                                                        c=DH + 1)
            nc.gpsimd.memset(vv[:, :, :, DH : DH + 1], 1.0)
            nc.gpsimd.memset(v_sb[:, :, H * (DH + 1) :], 0.0)
            # last key block only has 64 valid token rows; zero the rest
            nc.gpsimd.memset(v_sb[DH:P, NSS - 1, :], 0.0)
            for dv in range(2):
                wt = wp.tile([P, KS, 512], BF16, tag="wv", bufs=1)
                nc.sync.dma_start(wt[:], io["swv"][:, :, bass.ts(dv, 512)])
                for sc in range(NSS):
                    n_s = min(P, HALO + TCH - sc * P)
                    pt = ps.tile([P, 512], F32, tag="ps")
                    for k in range(KS):
                        _mm(nc, pt[:n_s, :], xb_sb[:, k, sc * P : sc * P + n_s],
                            wt[:, k, :], k == 0, k == KS - 1)
                    dst = v_sb[:n_s, sc,
                               dv * 8 * (DH + 1) : (dv * 8 + 8) * (DH + 1)]
                    dst = dst.rearrange("p (h c) -> p h c", c=DH + 1)[:, :, 0:DH]
                    nc.vector.tensor_copy(
                        out=dst,
                        in_=pt[:n_s, :].rearrange("p (h c) -> p h c", c=DH))

            # cross-attn K for group 0: independent matmul work the scheduler
            # can use to fill PE gaps during the self-attention inner loop
            k8_g0 = kv_k(0)

            av2 = avp.tile([P, KS, TCH], BF16, tag="av2")
            for j in range(H // 2):
                he, ho = 2 * j, 2 * j + 1
                pav_e = ps.tile([P, 512], F32, tag="pav", bufs=3)
                pav_o = ps.tile([P, 512], F32, tag="pav", bufs=3)
                for tcn in range(4):
                    T0 = tcn * P
                    q_eb = qe[:, j, T0 : T0 + P]
                    q_ob = qo[:, j, T0 : T0 + P]
                    kA = k_sb[:, j, T0 : T0 + P]
                    kB = k_sb[:, j, T0 + P : T0 + 2 * P]
                    pS = ps.tile([P, 512], F32, tag="ps")
                    _mm(nc, pS[:, 0:P], kA, q_eb, True, True)
                    _mm(nc, pS[:, P : 2 * P], kB, q_eb, True, True)
                    _mm(nc, pS[:, 2 * P : 3 * P], kA, q_ob, True, True)
                    _mm(nc, pS[:, 3 * P :], kB, q_ob, True, True)
                    tp = tp0 if tcn == 0 else tp1
                    ef = epo.tile([P, 512], F32, tag="ef")
                    nc.vector.tensor_tensor(ef[:], pS[:], tp[:, j, :], ALU.add)
                    e = epo.tile([P, 512], BF16, tag="eb")
                    nc.scalar.activation(e[:], ef[:], AF.Exp)
                    we = he * (DH + 1)
                    wo = ho * (DH + 1)
                    _mm(nc, pav_e[:, T0 : T0 + P],
                        v_sb[:, tcn, we : we + P], e[:, 0:P], True, False)
                    _mm(nc, pav_e[:, T0 : T0 + P],
                        v_sb[:, tcn + 1, we : we + P], e[:, P : 2 * P],
                        False, True)
                    _mm(nc, pav_o[:, T0 : T0 + P],
                        v_sb[:, tcn, wo : wo + P], e[:, 2 * P : 3 * P],
                        True, False)
                    _mm(nc, pav_o[:, T0 : T0 + P],
                        v_sb[:, tcn + 1, wo : wo + P], e[:, 3 * P :],
                        False, True)
                _head_norm(nc, sm, pav_e, he, av2)
                _head_norm(nc, sm, pav_o, ho, av2)

            _oproj(nc, ps, wp, sm, av2, io, "swo", sb["sbo"], xf_sb)

        cp_stack = contextlib.ExitStack()
        cp2 = cp_stack.enter_context(tc.tile_pool(name="cp2", bufs=1))
        epc = cp_stack.enter_context(tc.tile_pool(name="expc", bufs=3))
        v8_g0 = kv_v(0, cp2)

        x1_sb = residp.tile([P, KS, TCH], F32, tag="resid")
        x1b = xqp.tile([P, KS, TCH], BF16, tag="xq")
        _ln(nc, ps, sm, xf_sb, x1_sb, sb["g1"], sb["be1"], ones128, eps_t,
            out_bf=x1b)

        # ================= CROSS-ATTENTION =================
        for do in range(KS):
            wt = wp.tile([P, KS, P], BF16, tag="wproj")
            nc.sync.dma_start(wt[:], io["cwq"][:, :, bass.ts(do, P)])
            pt = ps.tile([P, 512], F32, tag="ps")
            for k in range(KS):
                _mm(nc, pt[:], wt[:, k, :], x1b[:, k, :], k == 0, k == KS - 1)
            nc.scalar.activation(qe[0:DH, do, :], pt[0:DH, :], AF.Identity,
                                 bias=sb["cbq"][0:DH, do : do + 1], scale=1.0)
            nc.vector.tensor_scalar(out=qo[DH:P, do, :], in0=pt[DH:P, :],
                                    scalar1=sb["cbq"][DH:P, do : do + 1],
                                    scalar2=None, op0=ALU.add)

        av2 = avp.tile([P, KS, TCH], BF16, tag="av2")
        kv = (k8_g0, v8_g0)
        for g in range(2):
            k8, v8 = kv
            for j in range(4):
                he, ho = g * 8 + 2 * j, g * 8 + 2 * j + 1
                q_e = qe[:, g * 4 + j, :]
                q_o = qo[:, g * 4 + j, :]
                pav_e = ps.tile([P, 512], F32, tag="pav", bufs=3)
                pav_o = ps.tile([P, 512], F32, tag="pav", bufs=3)
                for sc in range(NSC):
                    pSe = ps.tile([P, 512], F32, tag="ps")
                    pSo = ps.tile([P, 512], F32, tag="ps")
                    _mm(nc, pSe[:], k8[:, j, bass.ts(sc, P)], q_e, True, True)
                    _mm(nc, pSo[:], k8[:, j, bass.ts(sc, P)], q_o, True, True)
                    eSe = epc.tile([P, TCH], BF16, tag="ec")
                    eSo = epc.tile([P, TCH], BF16, tag="ec2")
                    nc.scalar.activation(eSe[:], pSe[:], AF.Exp)
                    nc.scalar.activation(eSo[:], pSo[:], AF.Exp)
                    vA_e = v8[:, sc, (2 * j) * (DH + 1) : (2 * j) * (DH + 1) + P]
                    vA_o = v8[:, sc,
                              (2 * j + 1) * (DH + 1) : (2 * j + 1) * (DH + 1) + P]
                    _mm(nc, pav_e[:], vA_e, eSe, sc == 0, sc == NSC - 1)
                    _mm(nc, pav_o[:], vA_o, eSo, sc == 0, sc == NSC - 1)
                _head_norm(nc, sm, pav_e, he, av2)
                _head_norm(nc, sm, pav_o, ho, av2)
                if g == 0 and j == 3:
                    kv = (kv_k(1), kv_v(1, cp2))

        _oproj(nc, ps, wp, sm, av2, io, "cwo", sb["cbo"], x1_sb)
        cp_stack.close()
        kp_stack.close()

        x2_sb = residp.tile([P, KS, TCH], F32, tag="resid")
        x2b = xqp.tile([P, KS, TCH], BF16, tag="xq")
        _ln(nc, ps, sm, x1_sb, x2_sb, sb["g2"], sb["be2"], ones128, eps_t,
            out_bf=x2b)

        # ================= FFN =================
        with tc.tile_pool(name="ffnp", bufs=1) as fp, \
             tc.tile_pool(name="w2p", bufs=2) as w2p:
            h1 = fp.tile([P, NFF, TCH], BF16)
            for fc in range(NFF):
                wt = wp.tile([P, KS, P], BF16, tag="wproj")
                nc.sync.dma_start(wt[:], io["w1"][:, :, bass.ts(fc, P)])
                pt = ps.tile([P, 512], F32, tag="ps")
                for k in range(KS):
                    _mm(nc, pt[:], wt[:, k, :], x2b[:, k, :], k == 0, k == KS - 1)
                nc.scalar.activation(h1[:, fc, :], pt[:], AF.Gelu,
                                     bias=sb["b1"][:, fc : fc + 1], scale=1.0)
            for do in range(KS):
                wt = w2p.tile([P, NFF, P], BF16, tag="w2t")
                nc.sync.dma_start(wt[:], io["w2"][:, :, bass.ts(do, P)])
                pt = ps.tile([P, 512], F32, tag="ps")
                for k in range(NFF):
                    _mm(nc, pt[:], wt[:, k, :], h1[:, k, :], k == 0, k == NFF - 1)
                ft = sm.tile([P, TCH], F32, tag="t2k")
                nc.vector.tensor_scalar(out=ft[:], in0=pt[:],
                                        scalar1=sb["b2"][:, do : do + 1],
                                        scalar2=None, op0=ALU.add)
                nc.vector.tensor_tensor(x2_sb[:, do, :], ft[:], x2_sb[:, do, :],
                                        ALU.add)

        out_sb = residp.tile([P, KS, TCH], F32, tag="resid")
        _ln(nc, ps, sm, x2_sb, out_sb, sb["g3"], sb["be3"], ones128, eps_t)
        nc.sync.dma_start(io["out"][:], out_sb[:])


def _head_norm(nc, sm, pav, h, av2):
    """Softmax-normalize one head's AV block (rows 0:64 of pav, row sums in
    row 64) straight into its pair-stacked position in av2. Fully on-chip:
    1/x = exp(-ln(x)) on ACT, partition broadcast on gpsimd."""
    rs = sm.tile([1, TCH], F32, tag="rsrow", bufs=3)
    nc.vector.tensor_copy(out=rs[0:1, :], in_=pav[DH : DH + 1, :])
    rln = sm.tile([1, TCH], F32, tag="rln", bufs=2)
    nc.scalar.activation(rln[:], rs[:], AF.Ln)
    rcp_row = sm.tile([1, TCH], BF16, tag="rcprow", bufs=2)
    with nc.allow_low_precision(reason="softmax denominator in bf16"):
        nc.scalar.activation(rcp_row[:], rln[:], AF.Exp, scale=-1.0)
    rcp = sm.tile([DH, TCH], BF16, tag="rcph", bufs=3)
    nc.gpsimd.partition_broadcast(rcp[:], rcp_row[0:1, :])
    nc.vector.tensor_tensor(av2[(h % 2) * DH : ((h % 2) + 1) * DH, h // 2, :],
                            pav[0:DH, :], rcp[:], ALU.mult)


def _oproj(nc, ps, wp, sm, av2, io, wo_name, bo_sb, res_sb):
    """Standard K=128 projection of the pair-stacked av2 with wo in natural
    layout; bias + residual add into res_sb in place."""
    for do in range(KS):
        wt = wp.tile([P, KS, P], BF16, tag="wproj")
        nc.sync.dma_start(wt[:], io[wo_name][:, :, bass.ts(do, P)])
        pt = ps.tile([P, 512], F32, tag="ps")
        for k in range(KS):
            _mm(nc, pt[:], wt[:, k, :], av2[:, k, :], k == 0, k == KS - 1)
        st = sm.tile([P, TCH], F32, tag="t2k")
        nc.vector.tensor_scalar(out=st[:], in0=pt[:],
                                scalar1=bo_sb[:, do : do + 1], scalar2=None,
                                op0=ALU.add)
        nc.vector.tensor_tensor(res_sb[:, do, :], st[:], res_sb[:, do, :], ALU.add)


def _ln(nc, ps, sm, src_sb, out_sb, g, be, ones128, eps_t, out_bf=None):
    """out = LN(src) over the feature (partition) axis, feature-major.
    Sum/sumsq via full-tile ones matmuls (row 0 of psum); mean/rstd
    broadcast across partitions on gpsimd."""
    psum = ps.tile([P, 512], F32, tag="ps")
    psq = ps.tile([P, 512], F32, tag="ps")
    for k in range(KS):
        xr = sm.tile([P, TCH], F32R, tag="lnxr", bufs=1)
        nc.vector.tensor_copy(out=xr[:], in_=src_sb[:, k, :])
        _mm(nc, psum[:], ones128, xr[:], k == 0, k == KS - 1)
        sq = sm.tile([P, TCH], F32R, tag="lnsq", bufs=1)
        nc.vector.tensor_tensor(sq[:], xr[:], xr[:], ALU.mult)
        _mm(nc, psq[:], ones128, sq[:], k == 0, k == KS - 1)
    mean = sm.tile([1, TCH], F32, tag="lnm", bufs=1)
    ex2 = sm.tile([1, TCH], F32, tag="lnv", bufs=1)
    nc.vector.tensor_scalar(out=mean[:], in0=psum[:1, :], scalar1=1.0 / D,
                            scalar2=None, op0=ALU.mult)
    nc.vector.tensor_scalar(out=ex2[:], in0=psq[:1, :], scalar1=1.0 / D,
                            scalar2=None, op0=ALU.mult)
    var = sm.tile([1, TCH], F32, tag="lnvar", bufs=1)
    nc.vector.tensor_tensor(var[:], mean[:], mean[:], ALU.mult)
    nc.vector.tensor_tensor(var[:], ex2[:], var[:], ALU.subtract)
    rstd = sm.tile([1, TCH], F32, tag="lnstd", bufs=1)
    # rsqrt(v) = exp(-0.5*ln(v+eps)); Rsqrt itself is blocked on ACT
    nc.scalar.activation(rstd[:], var[:], AF.Ln, bias=eps_t[:1, :], scale=1.0)
    nc.scalar.activation(rstd[:], rstd[:], AF.Exp, scale=-0.5)
    mb = sm.tile([P, TCH], F32, tag="lnb")
    rb = sm.tile([P, TCH], F32, tag="lnb")
    nc.gpsimd.partition_broadcast(mb[:], mean[0:1, :])
    nc.gpsimd.partition_broadcast(rb[:], rstd[0:1, :])
    for k in range(KS):
        nc.vector.tensor_tensor(out_sb[:, k, :], src_sb[:, k, :], mb[:],
                                ALU.subtract)
        nc.vector.tensor_tensor(out_sb[:, k, :], out_sb[:, k, :], rb[:], ALU.mult)
        nc.vector.tensor_scalar(out=out_sb[:, k, :], in0=out_sb[:, k, :],
                                scalar1=g[:, k : k + 1], scalar2=be[:, k : k + 1],
                                op0=ALU.mult, op1=ALU.add)
        if out_bf is not None:
            nc.vector.tensor_copy(out=out_bf[:, k, :], in_=out_sb[:, k, :])


# ======================= host side =======================

_CACHE = {}


def _fm(a):
    """[T, D] -> feature-major [128, KS, T]."""
    T = a.shape[0]
    return np.ascontiguousarray(a.T.reshape(KS, P, T).transpose(1, 0, 2))


def _wfm(w):
    """[D_in, D_out] -> [128, D_in//128, D_out]."""
    return np.ascontiguousarray(w.reshape(-1, P, w.shape[1]).transpose(1, 0, 2))


def _pbias(b):
    return np.ascontiguousarray(b.reshape(-1, P).T.astype(np.float32))


def _templates():
    """Per-pair additive score templates [sr, pair, 4*128]:
    [A(even) | B(even) | A(odd) | B(odd)] along the last axis."""
    slopes = (2.0 ** (-8.0 * np.arange(1, H + 1) / H)).astype(np.float64)
    sr = np.arange(P)[:, None]
    tr = np.arange(P)[None, :]
    dA = sr - tr
    dB = sr + P - tr
    A = np.where((dA >= 1) & (dA <= 64),
                 (dA - 64)[None] * slopes[:, None, None], NEG)
    Bt = np.where((dB >= 1) & (dB <= 64),
                  (dB - 64)[None] * slopes[:, None, None], NEG)
    A0 = A.copy()
    A0[:, :64, :] = NEG

    def pack(Ax):
        blocks = [np.concatenate([Ax[2 * j], Bt[2 * j], Ax[2 * j + 1],
                                  Bt[2 * j + 1]], axis=1)
                  for j in range(H // 2)]  # each [sr, 512]
        t = np.stack(blocks, axis=0)  # [8, sr, 512]
        return np.ascontiguousarray(t.transpose(1, 0, 2).astype(np.float32))

    return pack(A), pack(A0)


def kernel(**inputs):
    bf = ml_dtypes.bfloat16
    x = np.asarray(inputs["x"], np.float32)
    mem = np.asarray(inputs["mem"], np.float32)
    g = lambda n: np.asarray(inputs[n], np.float32)

    tP, tP0 = _templates()
    CH = L // TCH

    bias_list = {
        "sbq": g("sbq") / 8.0, "sbk": g("sbk"),
        "sbo": g("sbo") + g("sbv") @ g("swo"), "cbq": g("cbq") / 8.0,
        "cbk": g("cbk"), "cbo": g("cbo") + g("cbv") @ g("cwo"),
        "b2": g("b2"),
        "g1": g("g1"), "be1": g("be1"), "g2": g("g2"), "be2": g("be2"),
        "g3": g("g3"), "be3": g("be3"),
    }
    biases = np.concatenate([_pbias(bias_list[n]) for n in NBIAS]
                            + [_pbias(g("b1"))], axis=1)

    shared = {
        "swq": _wfm(g("swq") / 8.0).astype(bf), "swk": _wfm(g("swk")).astype(bf),
        "swv": _wfm(g("swv")).astype(bf), "swo": _wfm(g("swo")).astype(bf),
        "cwq": _wfm(g("cwq") / 8.0).astype(bf), "cwk": _wfm(g("cwk")).astype(bf),
        "cwv": _wfm(g("cwv")).astype(bf), "cwo": _wfm(g("cwo")).astype(bf),
        "w1": _wfm(g("w1")).astype(bf), "w2": _wfm(g("w2")).astype(bf),
        "biases": np.ascontiguousarray(biases),
        "tmplP": tP.astype(bf),
    }
    mem_fm = [np.ascontiguousarray(_fm(mem[b]).astype(bf)) for b in range(B)]

    in_maps = []
    for core in range(8):
        b, c = core // CH, core % CH
        t0 = c * TCH
        xpad = np.zeros((HALO + TCH, D), np.float32)
        lo = max(0, t0 - HALO)
        xpad[HALO - (t0 - lo):] = x[b, lo : t0 + TCH]
        m = dict(shared)
        m["memf"] = mem_fm[b]
        m["xb"] = np.ascontiguousarray(_fm(xpad).astype(bf))
        m["xf"] = np.ascontiguousarray(_fm(x[b, t0 : t0 + TCH]))
        m["tmplP0"] = (tP0 if c == 0 else tP).astype(bf)
        in_maps.append(m)

    if "nc" not in _CACHE:
        nc = bacc.Bacc("TRN2", target_bir_lowering=False, debug=False,
                       num_devices=8)
        build(nc)
        nc.compile()
        _CACHE["nc"] = nc
    nc = _CACHE["nc"]

    res = run_bass_kernel_spmd(nc, in_maps, core_ids=list(range(8)),
                               **_CACHE.get("run_kwargs", {}))
    _CACHE["last"] = res

    y = np.empty((B, L, D), np.float32)
    for core in range(8):
        b, c = core // CH, core % CH
        o = np.asarray(res.results[core]["out"])  # [128, KS, TCH]
        y[b, c * TCH : (c + 1) * TCH, :] = o.transpose(1, 0, 2).reshape(D, TCH).T
    return y
